# revision 1
# baseline (speedup 1.0000x reference)
"""Capsule-routing kernel for Trainium2 (8 NeuronCores, Bass/Tile).

Problem: u[b,o,k,j] = sum_i x[b,k,i] W[o,k,i,j]; 3 iters of dynamic routing
(softmax over o of per-(b,k) logits, squash over j), output v[b,o,j].

Sharding: input-capsule (IC=2048) dim split across 8 cores (256 each).
Per core: u (8M elems) is produced by TensorE matmuls with a block-diagonal
x as the stationary operand and W streamed, kept resident in SBUF as bf16
in layout [p=(b16,k8), kg32, (o,j)1024] (two tensors, one per batch-half).
Routing weighted-sums over k run as (DVE multiply) + (TensorE delta-ones
reduction with PSUM accumulation); the per-(b,o,k) logit update (sum over j
of a*u) runs as DVE multiply + contiguous-halves add tree. The per-iteration
partial s [32,1024] is AllReduce'd across the 8 cores (routing softmax is
local: o lives in the free dim).

Shapes (hardcoded): B=32, IC=2048, IV=16, OC=32, OV=32, T=3.
"""

import sys

sys.path.insert(0, "/opt/trn_rl_repo")

import numpy as np
import ml_dtypes

import concourse.bass as bass
import concourse.mybir as mybir
import concourse.tile as tile

BF16 = mybir.dt.bfloat16
F32 = mybir.dt.float32

NCORES = 8
B, IC, IV, OC, OV = 32, 2048, 16, 32, 32
KL = IC // NCORES          # 256 local input capsules per core
KG = KL // 8               # 32 k-groups of 8
OJ = OC * OV               # 1024
T = 3

_BF = ml_dtypes.bfloat16


def build_nc():
    nc = bass.Bass()

    w_in = nc.declare_dram_parameter("w", [KG, 128, OJ], BF16, isOutput=False)
    xblk_in = nc.declare_dram_parameter("xblk", [128, 2, KG, 128], BF16, isOutput=False)
    dsum0_in = nc.declare_dram_parameter("dsum0", [2, 128, 32], BF16, isOutput=False)
    dsumA_in = nc.declare_dram_parameter("dsumA", [2, 128, 32], BF16, isOutput=False)
    drep_in = nc.declare_dram_parameter("drep", [2, 32, 128], BF16, isOutput=False)
    v_out = nc.declare_dram_parameter("v", [B, OJ], F32, isOutput=True)

    with tile.TileContext(nc) as tc:
        with nc.allow_low_precision(reason="bf16 routing intermediates"):
            _emit(nc, tc, w_in, xblk_in, dsum0_in, dsumA_in, drep_in, v_out)
    _split_excess_waits(nc)
    return nc


def _split_excess_waits(nc):
    """The walrus build in this container accepts very few sync-wait commands
    per ISA struct (1 for DMA pseudo-instructions, 2 for compute engines).
    Tile attaches more. Move the excess onto same-engine NoOps inserted just
    before the instruction — NX executes the stream in order, so semantics
    are preserved (walrus accepts many waits on NoOp)."""
    ctr = 0
    for fn in nc.m.functions:
        for blk in fn.blocks:
            insts = blk.instructions
            idx = 0
            while idx < len(insts):
                inst = insts[idx]
                si = inst.sync_info
                if si is None or len(si.on_wait or []) <= 1:
                    idx += 1
                    continue
                waits = list(si.on_wait)
                for w in waits[:-1]:
                    carrier = mybir.InstNoOp(
                        name=f"I-wsplit-{ctr}",
                        sync_info=mybir.SyncInfo(on_wait=[w], on_update=[]),
                        bass_nofuse=True,
                        engine=inst.engine,
                    )
                    ctr += 1
                    blk.instructions.insert(idx, carrier)
                    idx += 1
                si.on_wait = waits[-1:]
                idx += 1


def _emit(nc, tc, w_in, xblk_in, dsum0_in, dsumA_in, drep_in, v_out):
    from contextlib import ExitStack

    ctx = ExitStack()
    with ctx:
        singles = ctx.enter_context(tc.tile_pool(name="singles", bufs=1))
        upool = ctx.enter_context(tc.tile_pool(name="u", bufs=1))
        dpool = ctx.enter_context(tc.tile_pool(name="dram", bufs=1, space="DRAM"))
        pmix = ctx.enter_context(tc.tile_pool(name="pmix", bufs=2, space="PSUM"))
        smalls = ctx.enter_context(tc.tile_pool(name="smalls", bufs=1))
        xbp = ctx.enter_context(tc.tile_pool(name="xb", bufs=1))
        wst = ctx.enter_context(tc.tile_pool(name="wst", bufs=8))
        ppu = ctx.enter_context(tc.tile_pool(name="ppu", bufs=3, space="PSUM"))
        pjk = ctx.enter_context(tc.tile_pool(name="pjk", bufs=1, space="PSUM"))
        m3p = ctx.enter_context(tc.tile_pool(name="m3p", bufs=2))
        m6p = ctx.enter_context(tc.tile_pool(name="m6p", bufs=1))
        trp = ctx.enter_context(tc.tile_pool(name="trp", bufs=1))
        t5p = ctx.enter_context(tc.tile_pool(name="t5p", bufs=1))
        smp = ctx.enter_context(tc.tile_pool(name="smp", bufs=1))

        # ---- resident tiles ----
        u_sb = [
            upool.tile([128, KG, OJ], BF16, tag=f"u{bh}", name=f"u{bh}") for bh in range(2)
        ]
        dsum0_sb = [singles.tile([128, 32], BF16, tag=f"ds0_{b}", name=f"ds0_{b}") for b in range(2)]
        dsumA_sb = [singles.tile([128, 32], BF16, tag=f"dsA_{b}", name=f"dsA_{b}") for b in range(2)]
        drep_sb = [singles.tile([32, 128], BF16, tag=f"drp_{b}", name=f"drp_{b}") for b in range(2)]
        blog = [singles.tile([128, KG, OC], BF16, tag=f"blog{bh}", name=f"blog{bh}") for bh in range(2)]
        c3 = [singles.tile([128, KG, OC], BF16, tag=f"c3_{bh}", name=f"c3_{bh}") for bh in range(2)]
        arep = [singles.tile([128, OJ], BF16, tag=f"arep{bh}", name=f"arep{bh}") for bh in range(2)]

        for bh in range(2):
            nc.sync.dma_start(dsum0_sb[bh], dsum0_in[bh])
            nc.sync.dma_start(dsumA_sb[bh], dsumA_in[bh])
            nc.sync.dma_start(drep_sb[bh], drep_in[bh])
            nc.vector.memset(blog[bh], 0.0)

        # ---- phase 1: produce u = x @ W (per k-group block-diag matmuls) ----
        xblk_sb = xbp.tile([128, 2, KG, 128], BF16)
        nc.sync.dma_start(xblk_sb, xblk_in[:])
        jxs = singles.tile([1, 128], BF16, tag="jxs", name="jxs")
        nc.sync.dma_start(jxs, w_in[0][0:1, 0:128])
        junk = pjk.tile([2, 8], F32, name="junk")
        for kg in range(KG):
            w4 = wst.tile([128, OJ], BF16, tag="w4", name="w4")
            nc.gpsimd.dma_start(w4, w_in[kg])
            # tiny matmul reads w4 first so the real matmuls don't carry the
            # DMA wait (MM ISA struct allows only 2 sync waits)
            nc.tensor.matmul(
                junk[:, 0:2],
                lhsT=xblk_sb[:, 0, kg, 0:2],
                rhs=w4[:, 0:2],
                start=True,
                stop=True,
            )
            for bh in range(2):
                for h in range(2):
                    pu = ppu.tile([128, 512], F32, tag="pu", name="pu")
                    nc.tensor.matmul(
                        pu,
                        lhsT=xblk_sb[:, bh, kg, :],
                        rhs=w4[:, 512 * h : 512 * h + 512],
                        start=True,
                        stop=True,
                    )
                    if bh == 0:
                        nc.vector.tensor_copy(
                            out=u_sb[bh][:, kg, 512 * h : 512 * h + 512],
                            in_=pu,
                        )
                    else:
                        nc.scalar.copy(
                            out=u_sb[bh][:, kg, 512 * h : 512 * h + 512],
                            in_=pu,
                        )

        # ---- helpers ----
        def weighted_sum_pass(use_c):
            """s_partial = sum_k (c * u) via DVE mult + delta-ones PE reduce."""
            ps = pmix.tile([B, OJ], F32, tag="pm", name="ps")
            # absorb the psum-slot-handoff + bank-guard waits so the real
            # accumulation-start matmul stays within the 2-sync-wait MM limit
            nc.tensor.matmul(
                ps[0:2, 0:2],
                lhsT=dsum0_sb[0][:, 0:2],
                rhs=dsum0_sb[0][:, 0:2],
                start=True,
                stop=True,
            )
            if True:
                for bh in range(2):
                    lhs = dsumA_sb[bh] if use_c else dsum0_sb[bh]
                    for kg in range(KG):
                        if use_c:
                            m3 = m3p.tile([128, OC, OV], BF16, tag="m3")
                            nc.vector.tensor_mul(
                                out=m3,
                                in0=u_sb[bh][:, kg].rearrange(
                                    "p (o j) -> p o j", j=OV
                                ),
                                in1=c3[bh][:, kg, :, None].to_broadcast(
                                    [128, OC, OV]
                                ),
                            )
                            rhs_full = m3.rearrange("p o j -> p (o j)")
                        else:
                            rhs_full = u_sb[bh][:, kg]
                        for h in range(2):
                            nc.tensor.matmul(
                                ps[:, 512 * h : 512 * h + 512],
                                lhsT=lhs,
                                rhs=rhs_full[:, 512 * h : 512 * h + 512],
                                start=(bh == 0 and kg == 0),
                                stop=(bh == 1 and kg == KG - 1),
                            )
            return ps

        def allreduce_s(ps, it):
            s_sb = smalls.tile([B, OJ], F32, tag="s_sb", name="s_sb")
            nc.vector.tensor_copy(out=s_sb, in_=ps)
            sp = dpool.tile([B, OJ], F32, tag=f"sp{it}", name=f"sp{it}")
            sf = dpool.tile([B, OJ], F32, tag=f"sf{it}", name=f"sf{it}")
            nc.gpsimd.dma_start(sp, s_sb)
            nc.gpsimd.collective_compute(
                "AllReduce",
                mybir.AluOpType.add,
                replica_groups=[list(range(NCORES))],
                ins=[sp.opt()],
                outs=[sf.opt()],
            )
            sf_sb = smalls.tile([B, OJ], F32, tag="sf_sb", name="sf_sb")
            nc.gpsimd.dma_start(sf_sb, sf)
            return sf_sb

        def squash(sf_sb, out_dtype, tag):
            sq = smalls.tile([B, OJ], F32, tag="sq", name="sq")
            nc.vector.tensor_mul(out=sq, in0=sf_sb, in1=sf_sb)
            n2 = smalls.tile([B, OC], F32, tag="n2", name="n2")
            nc.vector.reduce_sum(
                n2, sq.rearrange("b (o j) -> b o j", j=OV), axis=mybir.AxisListType.X
            )
            rt = smalls.tile([B, OC], F32, tag="rt", name="rt")
            nc.scalar.activation(rt, n2, mybir.ActivationFunctionType.Sqrt)
            dn = smalls.tile([B, OC], F32, tag="dn", name="dn")
            nc.vector.tensor_scalar_add(dn, n2, 1.0)
            nc.vector.reciprocal(dn, dn)
            f = smalls.tile([B, OC], F32, tag="f", name="f")
            nc.vector.tensor_mul(out=f, in0=rt, in1=dn)
            a_sb = smalls.tile([B, OC, OV], out_dtype, tag="av", name=tag)
            nc.vector.tensor_mul(
                out=a_sb,
                in0=sf_sb.rearrange("b (o j) -> b o j", j=OV),
                in1=f[:, :, None].to_broadcast([B, OC, OV]),
            )
            return a_sb

        def broadcast_a(a_sb):
            for bh in range(2):
                pr = pmix.tile([128, OJ], F32, tag="pm", name="pr")
                nc.tensor.matmul(
                    pr[0:2, 0:2],
                    lhsT=dsum0_sb[0][:, 0:2],
                    rhs=dsum0_sb[0][:, 0:2],
                    start=True,
                    stop=True,
                )
                af = a_sb.rearrange("b o j -> b (o j)")
                for h in range(2):
                    nc.tensor.matmul(
                        pr[:, 512 * h : 512 * h + 512],
                        lhsT=drep_sb[bh],
                        rhs=af[:, 512 * h : 512 * h + 512],
                        start=True,
                        stop=True,
                    )
                nc.scalar.copy(out=arep[bh], in_=pr)

        CH = 4  # kg per B-pass chunk

        def logit_update_pass():
            """blog += sum_j (a * u), via DVE mult + contiguous-halves tree."""
            if True:
                for bh in range(2):
                    for cc in range(KG // CH):
                        k0 = CH * cc
                        m6 = m6p.tile([128, CH, OJ], BF16, tag="m6", name="m6")
                        nc.vector.tensor_mul(
                            out=m6,
                            in0=u_sb[bh][:, k0 : k0 + CH],
                            in1=arep[bh][:, None, :].to_broadcast([128, CH, OJ]),
                        )
                        tt = trp.tile([128, CH, OC, 16], BF16, tag="tt", name="tt")
                        m6v = m6.rearrange("p c (o j) -> p c o j", j=OV)
                        # lvl1 -> tt[...,0:16]; lvl2 -> m6[...,0:8] (consumed);
                        # lvl3 -> tt[...,0:4] (lvl1 consumed); lvl4 -> tt[...,4:6]
                        nc.vector.tensor_add(
                            out=tt, in0=m6v[..., 0:16], in1=m6v[..., 16:32]
                        )
                        nc.vector.tensor_add(
                            out=m6v[..., 0:8], in0=tt[..., 0:8], in1=tt[..., 8:16]
                        )
                        nc.vector.tensor_add(
                            out=tt[..., 0:4], in0=m6v[..., 0:4], in1=m6v[..., 4:8]
                        )
                        nc.vector.tensor_add(
                            out=tt[..., 4:6], in0=tt[..., 0:2], in1=tt[..., 2:4]
                        )
                        t5 = t5p.tile([128, CH, OC], BF16, tag="t5", name="t5")
                        nc.vector.tensor_add(
                            out=t5[:, :, :, None],
                            in0=tt[..., 4:5],
                            in1=tt[..., 5:6],
                        )
                        nc.vector.tensor_add(
                            out=blog[bh][:, k0 : k0 + CH],
                            in0=blog[bh][:, k0 : k0 + CH],
                            in1=t5,
                        )

        def softmax_pass():
            if True:
                for bh in range(2):
                    nc.scalar.activation(
                        c3[bh], blog[bh], mybir.ActivationFunctionType.Exp
                    )
                    den = smp.tile([128, KG], F32, tag="den", name="den")
                    nc.vector.reduce_sum(den, c3[bh], axis=mybir.AxisListType.X)
                    nc.vector.reciprocal(den, den)
                    nc.vector.tensor_mul(
                        out=c3[bh],
                        in0=c3[bh],
                        in1=den[:, :, None].to_broadcast([128, KG, OC]),
                    )

        # ---- routing iterations ----
        for it in range(T):
            ps = weighted_sum_pass(use_c=(it > 0))
            sf_sb = allreduce_s(ps, it)
            if it < T - 1:
                a_sb = squash(sf_sb, BF16, tag="a_sb")
                broadcast_a(a_sb)
                logit_update_pass()
                softmax_pass()
            else:
                vt = squash(sf_sb, F32, tag="v_sb")
                nc.gpsimd.dma_start(v_out[:], vt.rearrange("b o j -> b (o j)"))


def _host_inputs(x, W):
    """Build per-core staged inputs (numpy, bf16) from full x [B,IC,IV], W [OC,IC,IV,OV]."""
    ins = []
    # constants, identical per core
    dsum0 = np.zeros((2, 128, 32), np.float32)
    dsumA = np.zeros((2, 128, 32), np.float32)
    drep = np.zeros((2, 32, 128), np.float32)
    for bh in range(2):
        for p in range(128):
            bl, k8 = p // 8, p % 8
            dsum0[bh, p, 16 * bh + bl] = 1.0 / OC
            dsumA[bh, p, 16 * bh + bl] = 1.0
            drep[bh, 16 * bh + bl, p] = 1.0
    dsum0 = dsum0.astype(_BF)
    dsumA = dsumA.astype(_BF)
    drep = drep.astype(_BF)

    for c in range(NCORES):
        ksl = slice(KL * c, KL * (c + 1))
        Wc = np.ascontiguousarray(W[:, ksl])  # [o, 256, i, j]
        # -> [kg, (k8 i), (o j)]
        wr = (
            Wc.reshape(OC, KG, 8, IV, OV)
            .transpose(1, 2, 3, 0, 4)
            .reshape(KG, 128, OJ)
            .astype(_BF)
        )
        xc = np.ascontiguousarray(x[:, ksl])  # [32, 256, 16]
        xr = xc.reshape(2, 16, KG, 8, IV)  # [bh, bl, kg, k8, i]
        xb = np.zeros((8, IV, 2, KG, 16, 8), np.float32)  # [k8,i,bh,kg,bl,k8']
        for k8 in range(8):
            xb[k8, :, :, :, :, k8] = xr[:, :, :, k8, :].transpose(3, 0, 2, 1)
        xblk = xb.reshape(128, 2, KG, 128).astype(_BF)
        ins.append(
            {
                "w": wr,
                "xblk": xblk,
                "dsum0": dsum0,
                "dsumA": dsumA,
                "drep": drep,
            }
        )
    return ins


def kernel(x: np.ndarray, W: np.ndarray) -> np.ndarray:
    from concourse.bass_utils import run_bass_kernel_spmd

    x = np.asarray(x, np.float32)
    W = np.asarray(W, np.float32)
    nc = build_nc()
    in_maps = _host_inputs(x, W)
    res = run_bass_kernel_spmd(nc, in_maps, core_ids=list(range(NCORES)))
    v = res.results[0]["v"].reshape(B, OC, OV).astype(np.float32)
    return v


if __name__ == "__main__":
    rng = np.random.default_rng(0)
    x = rng.standard_normal((B, IC, IV), dtype=np.float32)
    W = (0.01 * rng.standard_normal((OC, IC, IV, OV))).astype(np.float32)
    v = kernel(x, W)
    print("v", v.shape, v.dtype, float(np.abs(v).max()))



# revision 14
# speedup vs baseline: 3671.5939x; 3671.5939x over previous
"""Capsule-routing kernel for Trainium2 (8 NeuronCores, Bass/Tile).

Problem: u[b,o,k,j] = sum_i x[b,k,i] W[o,k,i,j]; 3 iters of dynamic routing
(softmax over o of per-(b,k) logits, squash over j), output v[b,o,j].

Sharding: input-capsule (IC=2048) dim split across 8 cores (256 each).
Per core: u (8M elems) is produced by TensorE matmuls with a block-diagonal
x as the stationary operand and W streamed, kept resident in SBUF as bf16
in layout [p=(b16,k8), kg32, (o,j)1024] (two tensors, one per batch-half).
Routing weighted-sums over k run as (DVE multiply) + (TensorE delta-ones
reduction with PSUM accumulation); the per-(b,o,k) logit update (sum over j
of a*u) runs as DVE multiply + contiguous-halves add tree. The per-iteration
partial s [32,1024] is AllReduce'd across the 8 cores (routing softmax is
local: o lives in the free dim).

Shapes (hardcoded): B=32, IC=2048, IV=16, OC=32, OV=32, T=3.
"""

import sys

sys.path.insert(0, "/opt/trn_rl_repo")

import numpy as np
import ml_dtypes

import concourse.bass as bass
import concourse.mybir as mybir
import concourse.tile as tile

BF16 = mybir.dt.bfloat16
F32 = mybir.dt.float32

NCORES = 8
B, IC, IV, OC, OV = 32, 2048, 16, 32, 32
KL = IC // NCORES          # 256 local input capsules per core
KG = KL // 8               # 32 k-groups of 8
OJ = OC * OV               # 1024
T = 3

_BF = ml_dtypes.bfloat16


def build_nc(split_waits=True):
    nc = bass.Bass()

    w_in = nc.declare_dram_parameter("w", [KG, 128, OJ], BF16, isOutput=False)
    xblk_in = nc.declare_dram_parameter("xblk", [128, 2, KG, 128], BF16, isOutput=False)
    xsum_in = nc.declare_dram_parameter("xsum", [128, KG, 32], BF16, isOutput=False)
    dsum0_in = nc.declare_dram_parameter("dsum0", [2, 128, 32], BF16, isOutput=False)
    dsumA_in = nc.declare_dram_parameter("dsumA", [2, 128, 32], BF16, isOutput=False)
    drep_in = nc.declare_dram_parameter("drep", [2, 32, 128], BF16, isOutput=False)
    v_out = nc.declare_dram_parameter("v", [B, OJ], F32, isOutput=True)

    with tile.TileContext(nc) as tc:
        with nc.allow_low_precision(reason="bf16 routing intermediates"):
            _emit(nc, tc, w_in, xblk_in, xsum_in, dsum0_in, dsumA_in, drep_in, v_out)
    if split_waits:
        _split_excess_waits(nc)
    return nc


def _split_excess_waits(nc):
    """The walrus build in this container accepts very few sync-wait commands
    per ISA struct (1 for DMA pseudo-instructions, 2 for compute engines).
    Tile attaches more. Move the excess onto same-engine NoOps inserted just
    before the instruction — NX executes the stream in order, so semantics
    are preserved (walrus accepts many waits on NoOp)."""
    ctr = 0
    for fn in nc.m.functions:
        for blk in fn.blocks:
            insts = blk.instructions
            idx = 0
            while idx < len(insts):
                inst = insts[idx]
                si = inst.sync_info
                if si is None or len(si.on_wait or []) <= 1:
                    idx += 1
                    continue
                waits = list(si.on_wait)
                for w in waits[:-1]:
                    carrier = mybir.InstNoOp(
                        name=f"I-wsplit-{ctr}",
                        sync_info=mybir.SyncInfo(on_wait=[w], on_update=[]),
                        bass_nofuse=True,
                        engine=inst.engine,
                    )
                    ctr += 1
                    blk.instructions.insert(idx, carrier)
                    idx += 1
                si.on_wait = waits[-1:]
                idx += 1


def _emit(nc, tc, w_in, xblk_in, xsum_in, dsum0_in, dsumA_in, drep_in, v_out):
    from contextlib import ExitStack

    ctx = ExitStack()
    with ctx:
        singles = ctx.enter_context(tc.tile_pool(name="singles", bufs=1))
        upool = ctx.enter_context(tc.tile_pool(name="u", bufs=1))
        dpool = ctx.enter_context(tc.tile_pool(name="dram", bufs=1, space="DRAM"))
        pmix = ctx.enter_context(tc.tile_pool(name="pmix", bufs=2, space="PSUM"))
        smalls = ctx.enter_context(tc.tile_pool(name="smalls", bufs=1))
        xbp = ctx.enter_context(tc.tile_pool(name="xb", bufs=1))
        wst = ctx.enter_context(tc.tile_pool(name="wst", bufs=5))
        ppu = ctx.enter_context(tc.tile_pool(name="ppu", bufs=3, space="PSUM"))
        pjk = ctx.enter_context(tc.tile_pool(name="pjk", bufs=1, space="PSUM"))
        m3p = ctx.enter_context(tc.tile_pool(name="m3p", bufs=2))
        m6p = ctx.enter_context(tc.tile_pool(name="m6p", bufs=1))
        trp = ctx.enter_context(tc.tile_pool(name="trp", bufs=1))
        t5p = ctx.enter_context(tc.tile_pool(name="t5p", bufs=1))
        smp = ctx.enter_context(tc.tile_pool(name="smp", bufs=1))

        # ---- resident tiles ----
        u_sb = [
            upool.tile([128, KG, OJ], BF16, tag=f"u{bh}", name=f"u{bh}") for bh in range(2)
        ]
        dsum0_sb = [singles.tile([128, 32], BF16, tag=f"ds0_{b}", name=f"ds0_{b}") for b in range(2)]
        dsumA_sb = [singles.tile([128, 32], BF16, tag=f"dsA_{b}", name=f"dsA_{b}") for b in range(2)]
        drep_sb = [singles.tile([32, 128], BF16, tag=f"drp_{b}", name=f"drp_{b}") for b in range(2)]
        blog = [singles.tile([128, KG, OC], BF16, tag=f"blog{bh}", name=f"blog{bh}") for bh in range(2)]
        # c (normalized) duplicated x2 along a trailing pair axis so the c*u
        # multiply's last AP dim is stride-1/n=2 -> DVE 2x mode
        c3e = [
            singles.tile([128, KG, OC, 2], BF16, tag=f"c3e{bh}", name=f"c3e{bh}")
            for bh in range(2)
        ]
        arep = [singles.tile([128, OJ], BF16, tag=f"arep{bh}", name=f"arep{bh}") for bh in range(2)]

        for bh in range(2):
            nc.sync.dma_start(dsum0_sb[bh], dsum0_in[bh])
            nc.sync.dma_start(dsumA_sb[bh], dsumA_in[bh])
            nc.sync.dma_start(drep_sb[bh], drep_in[bh])
            nc.vector.memset(blog[bh], 0.0)

        # ---- phase 1: produce u = x @ W (per k-group block-diag matmuls),
        # and fuse iteration-0's uniform weighted sum (c = 1/OC, folded into
        # xsum on the host) as two extra matmuls per kg on the same W stream.
        xblk_sb = xbp.tile([128, 2, KG, 128], BF16)
        nc.sync.dma_start(xblk_sb, xblk_in[:])
        xsum_sb = singles.tile([128, KG, 32], BF16, tag="xsum", name="xsum")
        nc.sync.dma_start(xsum_sb, xsum_in[:])
        jxs = singles.tile([1, 128], BF16, tag="jxs", name="jxs")
        nc.sync.dma_start(jxs, w_in[0][0:1, 0:128])
        junk = pjk.tile([2, 8], F32, name="junk")
        s0ps = pmix.tile([B, OJ], F32, tag="pm", name="s0ps")
        # absorb the xsum-DMA and s0ps-slot waits off the accumulation chain
        nc.tensor.matmul(
            s0ps[0:2, 0:2],
            lhsT=xsum_sb[:, 0, 0:2],
            rhs=xsum_sb[:, 0, 0:2],
            start=True,
            stop=True,
        )
        for kg in range(KG):
            w4 = wst.tile([128, OJ], BF16, tag="w4", name="w4")
            nc.gpsimd.dma_start(w4, w_in[kg])
            # tiny matmul reads w4 first so the real matmuls don't carry the
            # DMA wait (MM ISA struct allows only 2 sync waits)
            nc.tensor.matmul(
                junk[:, 0:2],
                lhsT=xblk_sb[:, 0, kg, 0:2],
                rhs=w4[:, 0:2],
                start=True,
                stop=True,
            )
            for bh in range(2):
                for h in range(2):
                    pu = ppu.tile([128, 512], F32, tag="pu", name="pu")
                    nc.tensor.matmul(
                        pu,
                        lhsT=xblk_sb[:, bh, kg, :],
                        rhs=w4[:, 512 * h : 512 * h + 512],
                        start=True,
                        stop=True,
                    )
                    if bh == 0:
                        nc.vector.tensor_copy(
                            out=u_sb[bh][:, kg, 512 * h : 512 * h + 512],
                            in_=pu,
                        )
                    else:
                        nc.scalar.copy(
                            out=u_sb[bh][:, kg, 512 * h : 512 * h + 512],
                            in_=pu,
                        )
            for h in range(2):
                nc.tensor.matmul(
                    s0ps[:, 512 * h : 512 * h + 512],
                    lhsT=xsum_sb[:, kg, :],
                    rhs=w4[:, 512 * h : 512 * h + 512],
                    start=(kg == 0),
                    stop=(kg == KG - 1),
                )

        # ---- helpers ----
        def weighted_sum_pass():
            """s_partial = sum_k (c * u) via DVE mult + delta-ones PE reduce.
            The multiply views the pair-duplicated c3e so its last AP dim is
            stride-1/n=2 (all operands bf16) -> DVE 2x perf mode."""
            ps = pmix.tile([B, OJ], F32, tag="pm", name="ps")
            # absorb the psum-slot-handoff + bank-guard waits so the real
            # accumulation-start matmul stays within the 2-sync-wait MM limit
            nc.tensor.matmul(
                ps[0:2, 0:2],
                lhsT=dsum0_sb[0][:, 0:2],
                rhs=dsum0_sb[0][:, 0:2],
                start=True,
                stop=True,
            )
            for bh in range(2):
                lhs = dsumA_sb[bh]
                for kg in range(KG):
                    m3 = m3p.tile([128, OC, OV], BF16, tag="m3")
                    nc.vector.tensor_mul(
                        out=m3.rearrange("p o (a b) -> p o a b", b=2),
                        in0=u_sb[bh][:, kg].rearrange(
                            "p (o a b) -> p o a b", o=OC, b=2
                        ),
                        in1=c3e[bh][:, kg, :, None, :].to_broadcast(
                            [128, OC, OV // 2, 2]
                        ),
                    )
                    rhs_full = m3.rearrange("p o j -> p (o j)")
                    for h in range(2):
                        nc.tensor.matmul(
                            ps[:, 512 * h : 512 * h + 512],
                            lhsT=lhs,
                            rhs=rhs_full[:, 512 * h : 512 * h + 512],
                            start=(bh == 0 and kg == 0),
                            stop=(bh == 1 and kg == KG - 1),
                        )
            return ps

        def allreduce_s(ps, it):
            s_sb = smalls.tile([B, OJ], F32, tag="s_sb", name="s_sb")
            nc.vector.tensor_copy(out=s_sb, in_=ps)
            sp = dpool.tile([B, OJ], F32, tag=f"sp{it}", name=f"sp{it}")
            sf = dpool.tile([B, OJ], F32, tag=f"sf{it}", name=f"sf{it}")
            nc.gpsimd.dma_start(sp, s_sb)
            nc.gpsimd.collective_compute(
                "AllReduce",
                mybir.AluOpType.add,
                replica_groups=[list(range(NCORES))],
                ins=[sp.opt()],
                outs=[sf.opt()],
            )
            sf_sb = smalls.tile([B, OJ], F32, tag="sf_sb", name="sf_sb")
            nc.gpsimd.dma_start(sf_sb, sf)
            return sf_sb

        def squash(sf_sb, out_dtype, tag):
            sq = smalls.tile([B, OJ], F32, tag="sq", name="sq")
            nc.vector.tensor_mul(out=sq, in0=sf_sb, in1=sf_sb)
            n2 = smalls.tile([B, OC], F32, tag="n2", name="n2")
            nc.vector.reduce_sum(
                n2, sq.rearrange("b (o j) -> b o j", j=OV), axis=mybir.AxisListType.X
            )
            rt = smalls.tile([B, OC], F32, tag="rt", name="rt")
            nc.scalar.activation(rt, n2, mybir.ActivationFunctionType.Sqrt)
            dn = smalls.tile([B, OC], F32, tag="dn", name="dn")
            nc.vector.tensor_scalar_add(dn, n2, 1.0)
            nc.vector.reciprocal(dn, dn)
            f = smalls.tile([B, OC], F32, tag="f", name="f")
            nc.vector.tensor_mul(out=f, in0=rt, in1=dn)
            a_sb = smalls.tile([B, OC, OV], out_dtype, tag="av", name=tag)
            nc.vector.tensor_mul(
                out=a_sb,
                in0=sf_sb.rearrange("b (o j) -> b o j", j=OV),
                in1=f[:, :, None].to_broadcast([B, OC, OV]),
            )
            return a_sb

        def broadcast_a(a_sb):
            for bh in range(2):
                pr = pmix.tile([128, OJ], F32, tag="pm", name="pr")
                nc.tensor.matmul(
                    pr[0:2, 0:2],
                    lhsT=dsum0_sb[0][:, 0:2],
                    rhs=dsum0_sb[0][:, 0:2],
                    start=True,
                    stop=True,
                )
                af = a_sb.rearrange("b o j -> b (o j)")
                for h in range(2):
                    nc.tensor.matmul(
                        pr[:, 512 * h : 512 * h + 512],
                        lhsT=drep_sb[bh],
                        rhs=af[:, 512 * h : 512 * h + 512],
                        start=True,
                        stop=True,
                    )
                nc.scalar.copy(out=arep[bh], in_=pr)

        CH = 4  # kg per B-pass chunk

        def logit_update_pass():
            """blog += sum_j (a * u), via DVE mult + contiguous-halves tree."""
            if True:
                for bh in range(2):
                    for cc in range(KG // CH):
                        k0 = CH * cc
                        m6 = m6p.tile([128, CH, OJ], BF16, tag="m6", name="m6")
                        nc.vector.tensor_mul(
                            out=m6,
                            in0=u_sb[bh][:, k0 : k0 + CH],
                            in1=arep[bh][:, None, :].to_broadcast([128, CH, OJ]),
                        )
                        tt = trp.tile([128, CH, OC, 16], BF16, tag="tt", name="tt")
                        m6v = m6.rearrange("p c (o j) -> p c o j", j=OV)
                        # lvl1 -> tt[...,0:16]; lvl2 -> m6[...,0:8] (consumed);
                        # lvl3 -> tt[...,0:4] (lvl1 consumed); lvl4 -> tt[...,4:6]
                        nc.vector.tensor_add(
                            out=tt, in0=m6v[..., 0:16], in1=m6v[..., 16:32]
                        )
                        nc.vector.tensor_add(
                            out=m6v[..., 0:8], in0=tt[..., 0:8], in1=tt[..., 8:16]
                        )
                        nc.vector.tensor_add(
                            out=tt[..., 0:4], in0=m6v[..., 0:4], in1=m6v[..., 4:8]
                        )
                        nc.vector.tensor_add(
                            out=tt[..., 4:6], in0=tt[..., 0:2], in1=tt[..., 2:4]
                        )
                        t5 = t5p.tile([128, CH, OC], BF16, tag="t5", name="t5")
                        nc.vector.tensor_add(
                            out=t5[:, :, :, None],
                            in0=tt[..., 4:5],
                            in1=tt[..., 5:6],
                        )
                        nc.vector.tensor_add(
                            out=blog[bh][:, k0 : k0 + CH],
                            in0=blog[bh][:, k0 : k0 + CH],
                            in1=t5,
                        )

        def softmax_pass():
            # exp(blog) lands directly in the pair-duplicated c3e; the summed
            # pair-duplicate doubles the denominator, so fold a x2 into the
            # reciprocal before normalizing in place.
            for bh in range(2):
                nc.scalar.activation(
                    c3e[bh],
                    blog[bh][:, :, :, None].to_broadcast([128, KG, OC, 2]),
                    mybir.ActivationFunctionType.Exp,
                )
                den = smp.tile([128, KG], F32, tag="den", name="den")
                nc.vector.reduce_sum(
                    den,
                    c3e[bh].rearrange("p k o b -> p k (o b)"),
                    axis=mybir.AxisListType.X,
                )
                nc.vector.reciprocal(den, den)
                nc.vector.tensor_scalar_mul(den, den, 2.0)
                nc.vector.tensor_mul(
                    out=c3e[bh],
                    in0=c3e[bh],
                    in1=den[:, :, None, None].to_broadcast([128, KG, OC, 2]),
                )

        # ---- routing iterations (it0's s came from the fused phase-1 matmuls)
        for it in range(T):
            ps = s0ps if it == 0 else weighted_sum_pass()
            sf_sb = allreduce_s(ps, it)
            if it < T - 1:
                a_sb = squash(sf_sb, BF16, tag="a_sb")
                broadcast_a(a_sb)
                logit_update_pass()
                softmax_pass()
            else:
                vt = squash(sf_sb, F32, tag="v_sb")
                nc.gpsimd.dma_start(v_out[:], vt.rearrange("b o j -> b (o j)"))


def _host_inputs(x, W):
    """Build per-core staged inputs (numpy, bf16) from full x [B,IC,IV], W [OC,IC,IV,OV]."""
    ins = []
    # constants, identical per core
    dsum0 = np.zeros((2, 128, 32), np.float32)
    dsumA = np.zeros((2, 128, 32), np.float32)
    drep = np.zeros((2, 32, 128), np.float32)
    for bh in range(2):
        for p in range(128):
            bl, k8 = p // 8, p % 8
            dsum0[bh, p, 16 * bh + bl] = 1.0 / OC
            dsumA[bh, p, 16 * bh + bl] = 1.0
            drep[bh, 16 * bh + bl, p] = 1.0
    dsum0 = dsum0.astype(_BF)
    dsumA = dsumA.astype(_BF)
    drep = drep.astype(_BF)

    for c in range(NCORES):
        ksl = slice(KL * c, KL * (c + 1))
        Wc = np.ascontiguousarray(W[:, ksl])  # [o, 256, i, j]
        # -> [kg, (k8 i), (o j)]
        wr = (
            Wc.reshape(OC, KG, 8, IV, OV)
            .transpose(1, 2, 3, 0, 4)
            .reshape(KG, 128, OJ)
            .astype(_BF)
        )
        xc = np.ascontiguousarray(x[:, ksl])  # [32, 256, 16]
        xr = xc.reshape(2, 16, KG, 8, IV)  # [bh, bl, kg, k8, i]
        xb = np.zeros((8, IV, 2, KG, 16, 8), np.float32)  # [k8,i,bh,kg,bl,k8']
        for k8 in range(8):
            xb[k8, :, :, :, :, k8] = xr[:, :, :, k8, :].transpose(3, 0, 2, 1)
        xblk = xb.reshape(128, 2, KG, 128).astype(_BF)
        # xsum[(k8,i), kg, b] = x[b, kg*8+k8, i] / OC  (iteration-0 uniform c)
        xsum = (
            (xr.transpose(3, 4, 2, 0, 1) / OC)  # [k8, i, kg, bh, bl]
            .reshape(128, KG, 32)
            .astype(_BF)
        )
        ins.append(
            {
                "w": wr,
                "xblk": xblk,
                "xsum": xsum,
                "dsum0": dsum0,
                "dsumA": dsumA,
                "drep": drep,
            }
        )
    return ins


def kernel(x: np.ndarray, W: np.ndarray) -> np.ndarray:
    from concourse.bass_utils import run_bass_kernel_spmd

    x = np.asarray(x, np.float32)
    W = np.asarray(W, np.float32)
    nc = build_nc()
    in_maps = _host_inputs(x, W)
    res = run_bass_kernel_spmd(nc, in_maps, core_ids=list(range(NCORES)))
    v = res.results[0]["v"].reshape(B, OC, OV).astype(np.float32)
    return v


if __name__ == "__main__":
    rng = np.random.default_rng(0)
    x = rng.standard_normal((B, IC, IV), dtype=np.float32)
    W = (0.01 * rng.standard_normal((OC, IC, IV, OV))).astype(np.float32)
    v = kernel(x, W)
    print("v", v.shape, v.dtype, float(np.abs(v).max()))

